# revision 12
# baseline (speedup 1.0000x reference)
"""Trainium2 Bass kernel for nn_Attention_12189117186326 (sparse causal attention).

Sharding: tensor-parallel over heads — 16 heads / 8 cores = 2 heads per core,
both batch elements on every core.  Per-core partial outputs (head-slice of the
output projection, head-sum of the attention matrix) are combined on the host.

Per-core math (heads h0=2c, h0+1), matmuls in float32r (PE-rounded fp32):
  qT,kT  [64, n]   = Wq/Wk-slice^T @ x^T          (scores layout, q pre-scaled)
  vN     [n, 65]   = (x @ Wv-slice | ones)        (ones column -> softmax denom)
  sT     [128j, 512i] = kT-block^T . qT-chunk     (transposed scores, causal trapezoid only)
  sT    += I^T . biasT-tile                       (fp16 identity-matmul adds pos-bias
                                                   + causal -30000 on the PE, not DVE)
  p      = exp(sT + keymask_j)                    (keymask via ACT per-partition bias)
  oT|den [65, 512i]  += vN-block^T . p            (row 64 = softmax denominator)
  attnT  += p * (1/den)                           (1/den broadcast via PE outer product)
  outp   [n, 1024] += oT-block^T . Wout-slice

All large DMA streams use tile-major DRAM layouts (one contiguous burst per
tile); the host packs/unpacks.
"""
import numpy as np

B, N, DIM, H, DH = 2, 2048, 1024, 16, 64
INNER = H * DH
N_CORES = 8
HPC = 2              # heads per core
P = 128              # partitions / j-block
CI = 512             # i-chunk width (one PSUM bank of fp32)
NCI = N // CI        # 4 i-chunks
NB = N // P          # 16 j-blocks
NEG = -30000.0       # mask additive constant (exp underflows to exactly 0)
KT = DIM // P        # k-tiles in the projection contractions

# trapezoid tile enumeration: (ci, jb) for jb covering j <= i
TILES = [(ci, jb) for ci in range(NCI) for jb in range((ci + 1) * (CI // P))]
NT = len(TILES)      # 40
TILE_IDX = {t: n for n, t in enumerate(TILES)}

_nc_cache = {}


def _build():
    import concourse.tile as tile
    from concourse import bacc, mybir

    f32 = mybir.dt.float32
    f32r = mybir.dt.float32r
    f16 = mybir.dt.float16

    nc = bacc.Bacc("TRN2", target_bir_lowering=False, debug=False, num_devices=N_CORES)

    xT_d = nc.dram_tensor("xT", [B, NCI, P, KT, CI], f32r, kind="ExternalInput").ap()
    wqk_d = nc.dram_tensor("wqk", [DIM, HPC, P], f32r, kind="ExternalInput").ap()
    wv_d = nc.dram_tensor("wv", [DIM, HPC * DH], f32r, kind="ExternalInput").ap()
    wout_d = nc.dram_tensor("wout", [HPC, DH, DIM], f32r, kind="ExternalInput").ap()
    eb_d = nc.dram_tensor("eb", [HPC, NT, P, CI], f16, kind="ExternalInput").ap()
    km_d = nc.dram_tensor("km", [B, NB, P], f32, kind="ExternalInput").ap()
    idf_d = nc.dram_tensor("idf", [P, P], f16, kind="ExternalInput").ap()
    onesr_d = nc.dram_tensor("onesr", [1, P], f32r, kind="ExternalInput").ap()
    onesv_d = nc.dram_tensor("onesv", [P, HPC, NB, 1], f32r, kind="ExternalInput").ap()
    outp_d = nc.dram_tensor("outp", [B, NB, DIM // CI, P, CI], f32, kind="ExternalOutput").ap()
    attnT_d = nc.dram_tensor("attnT", [B, NT, P, CI], f32r, kind="ExternalOutput").ap()

    from concourse.masks import make_identity

    with tile.TileContext(nc) as tc:
        with tc.tile_pool(name="w", bufs=1) as wpool:
            wqk_sb = wpool.tile([P, KT, HPC, P], f32r, tag="wqk")
            nc.sync.dma_start(out=wqk_sb, in_=wqk_d.rearrange("(k p) h m -> p k h m", p=P))
            wv_sb = wpool.tile([P, KT, HPC * DH], f32r, tag="wv")
            nc.sync.dma_start(out=wv_sb, in_=wv_d.rearrange("(k p) m -> p k m", p=P))
            wout_sb = wpool.tile([DH, HPC, DIM], f32r, tag="wout")
            nc.sync.dma_start(out=wout_sb, in_=wout_d.rearrange("h p d -> p h d"))
            km_sb = wpool.tile([P, B, NB], f32, tag="km")
            nc.sync.dma_start(out=km_sb, in_=km_d.rearrange("b j p -> p b j"))
            ident = wpool.tile([P, P], f32, tag="ident")
            make_identity(nc, ident)
            idf_sb = wpool.tile([P, P], f16, tag="idf")
            nc.sync.dma_start(out=idf_sb, in_=idf_d)
            # ones row living on partition 64 — matmul lhsT/rhs bases must match
            # the denominator row of the attention PSUM (partition DH=64)
            ones_t = wpool.tile([DH + 1, P], f32r, tag="ones")
            nc.sync.dma_start(out=ones_t[DH:DH + 1, :], in_=onesr_d)

            for b in range(B):
                with tc.tile_pool(name="qkv", bufs=1) as qkvpool:
                    qT_sb = qkvpool.tile([DH, HPC, N], f32r, tag="qT")
                    kT_sb = qkvpool.tile([DH, HPC, N], f32r, tag="kT")
                    vN_sb = qkvpool.tile([P, HPC, NB, DH + 1], f32r, tag="vN")
                    nc.sync.dma_start(out=vN_sb[:, :, :, DH:DH + 1], in_=onesv_d)
                    oT_sb = qkvpool.tile([DH, HPC, N], f32r, tag="oT")

                    # ---- phase 1: q/k/v projections for this batch ----
                    with tc.tile_pool(name="xt", bufs=2) as xpool, \
                         tc.tile_pool(name="psq", bufs=2, space="PSUM") as psq, \
                         tc.tile_pool(name="psv", bufs=2, space="PSUM") as psv, \
                         tc.tile_pool(name="pst", bufs=2, space="PSUM") as pstp, \
                         tc.tile_pool(name="vtmp", bufs=2) as vtpool:
                        for cc in range(NCI):
                            cs = cc * CI
                            xt = xpool.tile([P, KT, CI], f32r, tag="xt")
                            nc.sync.dma_start(out=xt, in_=xT_d[b, cc])
                            for hl in range(HPC):
                                for qk in range(2):  # 0 -> q rows, 1 -> k rows
                                    ps = psq.tile([DH, CI], f32, tag="psqk")
                                    for k in range(KT):
                                        nc.tensor.matmul(
                                            out=ps[:],
                                            lhsT=wqk_sb[:, k, hl, qk * DH:(qk + 1) * DH],
                                            rhs=xt[:, k, :],
                                            start=(k == 0), stop=(k == KT - 1))
                                    dst = qT_sb if qk == 0 else kT_sb
                                    nc.scalar.copy(out=dst[:, hl, cs:cs + CI], in_=ps[:])
                            # vT chunk [128(2h*64), 512] then transpose to natural
                            psvt = psv.tile([P, CI], f32, tag="psvt")
                            for k in range(KT):
                                nc.tensor.matmul(out=psvt[:], lhsT=wv_sb[:, k, :],
                                                 rhs=xt[:, k, :],
                                                 start=(k == 0), stop=(k == KT - 1))
                            vt = vtpool.tile([P, CI], f32, tag="vt")
                            nc.vector.tensor_copy(out=vt[:], in_=psvt[:])
                            for t in range(CI // P):
                                ptr = pstp.tile([P, P], f32, tag="ptr")
                                nc.tensor.transpose(ptr[:], vt[:, t * P:(t + 1) * P], ident[:])
                                jb = cc * (CI // P) + t
                                for hl in range(HPC):
                                    nc.vector.tensor_copy(
                                        out=vN_sb[:, hl, jb, 0:DH],
                                        in_=ptr[:, hl * DH:(hl + 1) * DH])

                    # ---- phase 2: attention + fused projection for this batch ----
                    with tc.tile_pool(name="eb", bufs=6) as ebpool, \
                         tc.tile_pool(name="pp", bufs=36) as ppool, \
                         tc.tile_pool(name="rc", bufs=4) as rcpool, \
                         tc.tile_pool(name="ob", bufs=4) as obpool, \
                         tc.tile_pool(name="pssc", bufs=2, space="PSUM") as pssc, \
                         tc.tile_pool(name="psat", bufs=2, space="PSUM") as psat, \
                         tc.tile_pool(name="psrb", bufs=2, space="PSUM") as psrb, \
                         tc.tile_pool(name="pspr", bufs=2, space="PSUM") as pspr:
                        for ci in range(NCI):
                            cs = ci * CI
                            nj = (ci + 1) * (CI // P)
                            acc = None
                            for hl in range(HPC):
                                pa = psat.tile([DH + 1, CI], f32, tag="pa")
                                ptiles = []
                                # software-pipelined: attnV for tile jb-1 is emitted
                                # between tile jb's scores and its exp, so the PE
                                # never waits on the ACT exp of the tile it just made
                                for jb in range(nj):
                                    ps = pssc.tile([P, CI], f32, tag="pssc")
                                    nc.tensor.matmul(
                                        out=ps[:],
                                        lhsT=kT_sb[:, hl, jb * P:(jb + 1) * P],
                                        rhs=qT_sb[:, hl, cs:cs + CI],
                                        start=True, stop=False)
                                    ebt = ebpool.tile([P, CI], f16, tag="eb")
                                    nc.sync.dma_start(out=ebt, in_=eb_d[hl, TILE_IDX[(ci, jb)]])
                                    nc.tensor.matmul(
                                        out=ps[:], lhsT=idf_sb[:], rhs=ebt[:],
                                        start=False, stop=True)
                                    if jb >= 1:
                                        nc.tensor.matmul(
                                            out=pa[:], lhsT=vN_sb[:, hl, jb - 1, :],
                                            rhs=ptiles[jb - 1][:],
                                            start=(jb == 1), stop=False)
                                    p = ppool.tile([P, CI], f32r, tag="p")
                                    nc.scalar.activation(
                                        out=p[:], in_=ps[:],
                                        func=mybir.ActivationFunctionType.Exp,
                                        bias=km_sb[:, b, jb:jb + 1], scale=1.0)
                                    ptiles.append(p)
                                nc.tensor.matmul(
                                    out=pa[:], lhsT=vN_sb[:, hl, nj - 1, :],
                                    rhs=ptiles[nj - 1][:],
                                    start=(nj == 1), stop=True)
                                # denominator -> reciprocal (stays on partition 64) ->
                                # broadcast to all 128 partitions via PE outer product
                                dn = rcpool.tile([DH + 1, CI], f32r, tag="dn")
                                with nc.allow_low_precision(reason="f32r is fp32-width"):
                                    nc.vector.reciprocal(dn[DH:DH + 1, :], pa[DH:DH + 1, :])
                                rcb = psrb.tile([P, CI], f32, tag="rcb")
                                nc.tensor.matmul(out=rcb[:], lhsT=ones_t[DH:DH + 1, :],
                                                 rhs=dn[DH:DH + 1, :], start=True, stop=True)
                                rcs = rcpool.tile([P, CI], f32, tag="rcs")
                                nc.scalar.copy(out=rcs[:], in_=rcb[:])
                                # normalized head output chunk
                                nc.vector.tensor_mul(
                                    oT_sb[:, hl, cs:cs + CI], pa[0:DH, :], rcs[0:DH, :])
                                # normalize p tiles; accumulate heads; store attnT
                                if hl == 0:
                                    acc = ptiles
                                    for jb in range(nj):
                                        nc.vector.tensor_mul(acc[jb][:], acc[jb][:], rcs[:])
                                else:
                                    for jb in range(nj):
                                        nc.vector.tensor_mul(ptiles[jb][:], ptiles[jb][:], rcs[:])
                                        nc.vector.tensor_add(acc[jb][:], acc[jb][:], ptiles[jb][:])
                                        nc.sync.dma_start(
                                            out=attnT_d[b, TILE_IDX[(ci, jb)]],
                                            in_=acc[jb][:])
                            # fused output projection for the n-blocks this ci completed
                            for nb in range(ci * (CI // P), (ci + 1) * (CI // P)):
                                for half in range(DIM // CI):
                                    ps = pspr.tile([P, CI], f32, tag="pspr")
                                    for hl in range(HPC):
                                        nc.tensor.matmul(
                                            out=ps[:],
                                            lhsT=oT_sb[:, hl, nb * P:(nb + 1) * P],
                                            rhs=wout_sb[:, hl, half * CI:(half + 1) * CI],
                                            start=(hl == 0), stop=(hl == HPC - 1))
                                    ob = obpool.tile([P, CI], f32, tag="ob")
                                    nc.scalar.copy(out=ob[:], in_=ps[:])
                                    nc.sync.dma_start(out=outp_d[b, nb, half], in_=ob[:])
    nc.compile()
    return nc


def _get_nc():
    if "nc" not in _nc_cache:
        _nc_cache["nc"] = _build()
    return _nc_cache["nc"]


def _prep_inputs(x, mask, positions_bias, W_qkv, W_out):
    scale = np.float32(DH ** -0.5)
    x = np.asarray(x, np.float32)
    mask = np.asarray(mask)
    pb = np.asarray(positions_bias, np.float32)
    W_qkv = np.asarray(W_qkv, np.float32)
    W_out = np.asarray(W_out, np.float32)

    # x tiled: xT_tiles[b, cc, p, k, n] = x[b, cc*CI+n, k*P+p]
    xT = np.ascontiguousarray(
        x.reshape(B, NCI, CI, KT, P).transpose(0, 1, 4, 3, 2))
    Wq, Wk, Wv = W_qkv[:, :INNER], W_qkv[:, INNER:2 * INNER], W_qkv[:, 2 * INNER:]
    km = np.where(mask, np.float32(NEG), np.float32(0.0)).astype(np.float32)
    km = np.ascontiguousarray(km.reshape(B, NB, P))

    # biasT[h, j, i] = pos_bias[h, i, j], NEG where j > i (causal); fp16, tile-major
    bT = pb[0].transpose(0, 2, 1)  # [H, j, i]
    tri = np.tri(N, dtype=bool).T  # [j, i], True where j <= i
    bT = np.where(tri[None, :, :], bT, np.float32(NEG)).astype(np.float16)
    eb_all = np.empty((H, NT, P, CI), np.float16)
    for t, (ci, jb) in enumerate(TILES):
        eb_all[:, t] = bT[:, jb * P:(jb + 1) * P, ci * CI:(ci + 1) * CI]

    in_maps = []
    for c in range(N_CORES):
        h0 = HPC * c
        wqk = np.empty((DIM, HPC, P), np.float32)
        for hl in range(HPC):
            h = h0 + hl
            wqk[:, hl, :DH] = Wq[:, h * DH:(h + 1) * DH] * scale
            wqk[:, hl, DH:] = Wk[:, h * DH:(h + 1) * DH]
        wv = np.ascontiguousarray(
            np.concatenate([Wv[:, (h0 + hl) * DH:(h0 + hl + 1) * DH] for hl in range(HPC)], axis=1))
        wout = np.ascontiguousarray(
            np.stack([W_out[(h0 + hl) * DH:(h0 + hl + 1) * DH, :] for hl in range(HPC)]))
        in_maps.append({"xT": xT, "wqk": wqk, "wv": wv, "wout": wout,
                        "eb": np.ascontiguousarray(eb_all[h0:h0 + HPC]), "km": km,
                        "idf": np.eye(P, dtype=np.float16),
                        "onesr": np.ones((1, P), np.float32),
                        "onesv": np.ones((P, HPC, NB, 1), np.float32)})
    return in_maps


def kernel(x, mask, positions_bias, W_qkv, W_out, b_out, _trace=False):
    from concourse.bass_utils import run_bass_kernel_spmd
    nc = _get_nc()
    in_maps = _prep_inputs(x, mask, positions_bias, W_qkv, W_out)
    res = run_bass_kernel_spmd(nc, in_maps, list(range(N_CORES)), trace=_trace)
    if _trace:
        _nc_cache["last_result"] = res
    outp_t = np.zeros((B, NB, DIM // CI, P, CI), np.float32)
    attnT_t = np.zeros((B, NT, P, CI), np.float32)
    for r in res.results:
        outp_t += r["outp"]
        attnT_t += r["attnT"]
    outp = outp_t.transpose(0, 1, 3, 2, 4).reshape(B, N, DIM)
    outp += np.asarray(b_out, np.float32)
    attn_avg = np.zeros((B, N, N), np.float32)
    for t, (ci, jb) in enumerate(TILES):
        attn_avg[:, ci * CI:(ci + 1) * CI, jb * P:(jb + 1) * P] = \
            attnT_t[:, t].transpose(0, 2, 1)
    attn_avg /= np.float32(H)
    return np.ascontiguousarray(outp), attn_avg


# revision 17
# speedup vs baseline: 1.0480x; 1.0480x over previous
"""Trainium2 Bass kernel for nn_Attention_12189117186326 (sparse causal attention).

Sharding: tensor-parallel over heads — 16 heads / 8 cores = 2 heads per core,
both batch elements on every core.  Per-core partial outputs (head-slice of the
output projection, head-sum of the attention matrix) are combined on the host.

Per-core math (heads h0=2c, h0+1), matmuls in float32r (PE-rounded fp32):
  qT,kT  [64, n]   = Wq/Wk-slice^T @ x^T          (scores layout, q pre-scaled)
  vN     [n, 65]   = (x @ Wv-slice | ones)        (ones column -> softmax denom)
  sT     [128j, 512i] = kT-block^T . qT-chunk     (transposed scores, causal trapezoid only)
  sT    += I^T . biasT-tile                       (fp16 identity-matmul adds pos-bias
                                                   + causal -30000 on the PE, not DVE)
  p      = exp(sT + keymask_j)                    (keymask via ACT per-partition bias)
  oT|den [65, 512i]  += vN-block^T . p            (row 64 = softmax denominator)
  attnT  += p * (1/den)                           (1/den broadcast via PE outer product)
  outp   [n, 1024] += oT-block^T . Wout-slice

All large DMA streams use tile-major DRAM layouts (one contiguous burst per
tile); the host packs/unpacks.
"""
import numpy as np

B, N, DIM, H, DH = 2, 2048, 1024, 16, 64
INNER = H * DH
N_CORES = 8
HPC = 2              # heads per core
P = 128              # partitions / j-block
CI = 512             # i-chunk width (one PSUM bank of fp32)
NCI = N // CI        # 4 i-chunks
NB = N // P          # 16 j-blocks
NEG = -30000.0       # mask additive constant (exp underflows to exactly 0)
KT = DIM // P        # k-tiles in the projection contractions

# trapezoid tile enumeration: (ci, jb) for jb covering j <= i
TILES = [(ci, jb) for ci in range(NCI) for jb in range((ci + 1) * (CI // P))]
NT = len(TILES)      # 40
TILE_IDX = {t: n for n, t in enumerate(TILES)}

_nc_cache = {}


def _build():
    import concourse.tile as tile
    from concourse import bacc, mybir

    f32 = mybir.dt.float32
    f32r = mybir.dt.float32r
    f16 = mybir.dt.float16

    nc = bacc.Bacc("TRN2", target_bir_lowering=False, debug=False, num_devices=N_CORES)

    xT_d = nc.dram_tensor("xT", [B, NCI, P, KT, CI], f32r, kind="ExternalInput").ap()
    wqk_d = nc.dram_tensor("wqk", [DIM, HPC, P], f32r, kind="ExternalInput").ap()
    wv_d = nc.dram_tensor("wv", [DIM, HPC * DH], f32r, kind="ExternalInput").ap()
    wout_d = nc.dram_tensor("wout", [HPC, DH, DIM], f32r, kind="ExternalInput").ap()
    eb_d = nc.dram_tensor("eb", [HPC, NT, P, CI], f16, kind="ExternalInput").ap()
    km_d = nc.dram_tensor("km", [B, NB, P], f32, kind="ExternalInput").ap()
    idf_d = nc.dram_tensor("idf", [P, P], f16, kind="ExternalInput").ap()
    onesr_d = nc.dram_tensor("onesr", [1, P], f32r, kind="ExternalInput").ap()
    onesv_d = nc.dram_tensor("onesv", [P, HPC, NB, 1], f32r, kind="ExternalInput").ap()
    outp_d = nc.dram_tensor("outp", [B, NB, DIM // CI, P, CI], f32, kind="ExternalOutput").ap()
    attnT_d = nc.dram_tensor("attnT", [B, NT, P, CI], f32r, kind="ExternalOutput").ap()

    from concourse.masks import make_identity

    with tile.TileContext(nc) as tc:
        with tc.tile_pool(name="w", bufs=1) as wpool:
            wqk_sb = wpool.tile([P, KT, HPC, P], f32r, tag="wqk")
            nc.sync.dma_start(out=wqk_sb, in_=wqk_d.rearrange("(k p) h m -> p k h m", p=P))
            wv_sb = wpool.tile([P, KT, HPC * DH], f32r, tag="wv")
            nc.sync.dma_start(out=wv_sb, in_=wv_d.rearrange("(k p) m -> p k m", p=P))
            wout_sb = wpool.tile([DH, HPC, DIM], f32r, tag="wout")
            nc.sync.dma_start(out=wout_sb, in_=wout_d.rearrange("h p d -> p h d"))
            km_sb = wpool.tile([P, B, NB], f32, tag="km")
            nc.sync.dma_start(out=km_sb, in_=km_d.rearrange("b j p -> p b j"))
            ident = wpool.tile([P, P], f32, tag="ident")
            make_identity(nc, ident)
            idf_sb = wpool.tile([P, P], f16, tag="idf")
            nc.sync.dma_start(out=idf_sb, in_=idf_d)
            # ones row living on partition 64 — matmul lhsT/rhs bases must match
            # the denominator row of the attention PSUM (partition DH=64)
            ones_t = wpool.tile([DH + 1, P], f32r, tag="ones")
            nc.sync.dma_start(out=ones_t[DH:DH + 1, :], in_=onesr_d)

            for b in range(B):
                with tc.tile_pool(name="qkv", bufs=1) as qkvpool:
                    qT_sb = qkvpool.tile([DH, HPC, N], f32r, tag="qT")
                    kT_sb = qkvpool.tile([DH, HPC, N], f32r, tag="kT")
                    vN_sb = qkvpool.tile([P, HPC, NB, DH + 1], f32r, tag="vN")
                    nc.sync.dma_start(out=vN_sb[:, :, :, DH:DH + 1], in_=onesv_d)
                    oT_sb = qkvpool.tile([DH, HPC, N], f32r, tag="oT")

                    # ---- phase 1: q/k/v projections for this batch ----
                    with tc.tile_pool(name="xt", bufs=2) as xpool, \
                         tc.tile_pool(name="psq", bufs=4, space="PSUM") as psq, \
                         tc.tile_pool(name="psv", bufs=2, space="PSUM") as psv, \
                         tc.tile_pool(name="pst", bufs=2, space="PSUM") as pstp, \
                         tc.tile_pool(name="vtmp", bufs=2) as vtpool:
                        for cc in range(NCI):
                            cs = cc * CI
                            xt = xpool.tile([P, KT, CI], f32r, tag="xt")
                            nc.sync.dma_start(out=xt, in_=xT_d[b, cc])
                            pss = {}
                            for hl in range(HPC):
                                for qk in range(2):
                                    pss[hl, qk] = psq.tile([DH, CI], f32, tag="psqk", name="psqk")
                            for k in range(KT):
                                for qk in range(2):
                                    for hl in range(HPC):
                                        nc.tensor.matmul(
                                            out=pss[hl, qk][:],
                                            lhsT=wqk_sb[:, k, hl, qk * DH:(qk + 1) * DH],
                                            rhs=xt[:, k, :],
                                            start=(k == 0), stop=(k == KT - 1))
                            for hl in range(HPC):
                                nc.scalar.copy(out=qT_sb[:, hl, cs:cs + CI], in_=pss[hl, 0][:])
                                nc.scalar.copy(out=kT_sb[:, hl, cs:cs + CI], in_=pss[hl, 1][:])
                            # vT chunk [128(2h*64), 512] then transpose to natural
                            psvt = psv.tile([P, CI], f32, tag="psvt")
                            for k in range(KT):
                                nc.tensor.matmul(out=psvt[:], lhsT=wv_sb[:, k, :],
                                                 rhs=xt[:, k, :],
                                                 start=(k == 0), stop=(k == KT - 1))
                            vt = vtpool.tile([P, CI], f32, tag="vt")
                            nc.vector.tensor_copy(out=vt[:], in_=psvt[:])
                            for t in range(CI // P):
                                ptr = pstp.tile([P, P], f32, tag="ptr")
                                nc.tensor.transpose(ptr[:], vt[:, t * P:(t + 1) * P], ident[:])
                                jb = cc * (CI // P) + t
                                for hl in range(HPC):
                                    nc.vector.tensor_copy(
                                        out=vN_sb[:, hl, jb, 0:DH],
                                        in_=ptr[:, hl * DH:(hl + 1) * DH])

                    # ---- phase 2: attention + fused projection for this batch ----
                    # the two heads' pipelines are interleaved instruction-by-
                    # instruction; their K=64 score matmuls occupy disjoint PE
                    # row-groups (partitions 0-63 vs 64-127) and run concurrently
                    with tc.tile_pool(name="eb", bufs=8) as ebpool, \
                         tc.tile_pool(name="pp", bufs=36) as ppool, \
                         tc.tile_pool(name="rc", bufs=4) as rcpool, \
                         tc.tile_pool(name="ob", bufs=4) as obpool, \
                         tc.tile_pool(name="pssc", bufs=3, space="PSUM") as pssc, \
                         tc.tile_pool(name="psat", bufs=2, space="PSUM") as psat, \
                         tc.tile_pool(name="psrb", bufs=1, space="PSUM") as psrb, \
                         tc.tile_pool(name="pspr", bufs=2, space="PSUM") as pspr:
                        for ci in range(NCI):
                            cs = ci * CI
                            nj = (ci + 1) * (CI // P)
                            pa = [psat.tile([DH + 1, CI], f32, tag="pa", name="pa") for _ in range(HPC)]
                            pt = [[], []]
                            # software-pipelined: attnV for tile jb-1 is emitted
                            # between tile jb's scores and its exp, so the PE
                            # never waits on the ACT exp of the tile it just made
                            for jb in range(nj):
                                pscs = []
                                for hl in range(HPC):
                                    ps = pssc.tile([P, CI], f32, tag="pssc")
                                    nc.tensor.matmul(
                                        out=ps[:],
                                        lhsT=kT_sb[:, hl, jb * P:(jb + 1) * P],
                                        rhs=qT_sb[:, hl, cs:cs + CI],
                                        start=True, stop=False)
                                    pscs.append(ps)
                                for hl in range(HPC):
                                    ebt = ebpool.tile([P, CI], f16, tag="eb")
                                    nc.sync.dma_start(out=ebt, in_=eb_d[hl, TILE_IDX[(ci, jb)]])
                                    nc.tensor.matmul(
                                        out=pscs[hl][:], lhsT=idf_sb[:], rhs=ebt[:],
                                        start=False, stop=True)
                                if jb >= 1:
                                    for hl in range(HPC):
                                        nc.tensor.matmul(
                                            out=pa[hl][:], lhsT=vN_sb[:, hl, jb - 1, :],
                                            rhs=pt[hl][jb - 1][:],
                                            start=(jb == 1), stop=False)
                                for hl in range(HPC):
                                    p = ppool.tile([P, CI], f32r, tag="p")
                                    nc.scalar.activation(
                                        out=p[:], in_=pscs[hl][:],
                                        func=mybir.ActivationFunctionType.Exp,
                                        bias=km_sb[:, b, jb:jb + 1], scale=1.0)
                                    pt[hl].append(p)
                            for hl in range(HPC):
                                nc.tensor.matmul(
                                    out=pa[hl][:], lhsT=vN_sb[:, hl, nj - 1, :],
                                    rhs=pt[hl][nj - 1][:],
                                    start=(nj == 1), stop=True)
                            # denominator -> reciprocal (stays on partition 64) ->
                            # broadcast to all 128 partitions via PE outer product
                            rcss = []
                            for hl in range(HPC):
                                dn = rcpool.tile([DH + 1, CI], f32r, tag="dn")
                                with nc.allow_low_precision(reason="f32r is fp32-width"):
                                    nc.vector.reciprocal(dn[DH:DH + 1, :], pa[hl][DH:DH + 1, :])
                                rcb = psrb.tile([P, CI], f32, tag="rcb")
                                nc.tensor.matmul(out=rcb[:], lhsT=ones_t[DH:DH + 1, :],
                                                 rhs=dn[DH:DH + 1, :], start=True, stop=True)
                                rcs = rcpool.tile([P, CI], f32, tag="rcs")
                                nc.scalar.copy(out=rcs[:], in_=rcb[:])
                                rcss.append(rcs)
                                # normalized head output chunk
                                nc.vector.tensor_mul(
                                    oT_sb[:, hl, cs:cs + CI], pa[hl][0:DH, :], rcs[0:DH, :])
                            # normalize p tiles; accumulate heads; store attnT
                            for jb in range(nj):
                                nc.vector.tensor_mul(pt[0][jb][:], pt[0][jb][:], rcss[0][:])
                                nc.vector.tensor_mul(pt[1][jb][:], pt[1][jb][:], rcss[1][:])
                                nc.vector.tensor_add(pt[0][jb][:], pt[0][jb][:], pt[1][jb][:])
                                nc.sync.dma_start(out=attnT_d[b, TILE_IDX[(ci, jb)]],
                                                  in_=pt[0][jb][:])
                            # fused output projection for the n-blocks this ci completed
                            for nb in range(ci * (CI // P), (ci + 1) * (CI // P)):
                                for half in range(DIM // CI):
                                    ps = pspr.tile([P, CI], f32, tag="pspr")
                                    for hl in range(HPC):
                                        nc.tensor.matmul(
                                            out=ps[:],
                                            lhsT=oT_sb[:, hl, nb * P:(nb + 1) * P],
                                            rhs=wout_sb[:, hl, half * CI:(half + 1) * CI],
                                            start=(hl == 0), stop=(hl == HPC - 1))
                                    ob = obpool.tile([P, CI], f32, tag="ob")
                                    nc.scalar.copy(out=ob[:], in_=ps[:])
                                    nc.sync.dma_start(out=outp_d[b, nb, half], in_=ob[:])
    nc.compile()
    return nc


def _get_nc():
    if "nc" not in _nc_cache:
        _nc_cache["nc"] = _build()
    return _nc_cache["nc"]


def _prep_inputs(x, mask, positions_bias, W_qkv, W_out):
    scale = np.float32(DH ** -0.5)
    x = np.asarray(x, np.float32)
    mask = np.asarray(mask)
    pb = np.asarray(positions_bias, np.float32)
    W_qkv = np.asarray(W_qkv, np.float32)
    W_out = np.asarray(W_out, np.float32)

    # x tiled: xT_tiles[b, cc, p, k, n] = x[b, cc*CI+n, k*P+p]
    xT = np.ascontiguousarray(
        x.reshape(B, NCI, CI, KT, P).transpose(0, 1, 4, 3, 2))
    Wq, Wk, Wv = W_qkv[:, :INNER], W_qkv[:, INNER:2 * INNER], W_qkv[:, 2 * INNER:]
    km = np.where(mask, np.float32(NEG), np.float32(0.0)).astype(np.float32)
    km = np.ascontiguousarray(km.reshape(B, NB, P))

    # biasT[h, j, i] = pos_bias[h, i, j], NEG where j > i (causal); fp16, tile-major
    bT = pb[0].transpose(0, 2, 1)  # [H, j, i]
    tri = np.tri(N, dtype=bool).T  # [j, i], True where j <= i
    bT = np.where(tri[None, :, :], bT, np.float32(NEG)).astype(np.float16)
    eb_all = np.empty((H, NT, P, CI), np.float16)
    for t, (ci, jb) in enumerate(TILES):
        eb_all[:, t] = bT[:, jb * P:(jb + 1) * P, ci * CI:(ci + 1) * CI]

    in_maps = []
    for c in range(N_CORES):
        h0 = HPC * c
        wqk = np.empty((DIM, HPC, P), np.float32)
        for hl in range(HPC):
            h = h0 + hl
            wqk[:, hl, :DH] = Wq[:, h * DH:(h + 1) * DH] * scale
            wqk[:, hl, DH:] = Wk[:, h * DH:(h + 1) * DH]
        wv = np.ascontiguousarray(
            np.concatenate([Wv[:, (h0 + hl) * DH:(h0 + hl + 1) * DH] for hl in range(HPC)], axis=1))
        wout = np.ascontiguousarray(
            np.stack([W_out[(h0 + hl) * DH:(h0 + hl + 1) * DH, :] for hl in range(HPC)]))
        in_maps.append({"xT": xT, "wqk": wqk, "wv": wv, "wout": wout,
                        "eb": np.ascontiguousarray(eb_all[h0:h0 + HPC]), "km": km,
                        "idf": np.eye(P, dtype=np.float16),
                        "onesr": np.ones((1, P), np.float32),
                        "onesv": np.ones((P, HPC, NB, 1), np.float32)})
    return in_maps


def kernel(x, mask, positions_bias, W_qkv, W_out, b_out, _trace=False):
    from concourse.bass_utils import run_bass_kernel_spmd
    nc = _get_nc()
    in_maps = _prep_inputs(x, mask, positions_bias, W_qkv, W_out)
    res = run_bass_kernel_spmd(nc, in_maps, list(range(N_CORES)), trace=_trace)
    if _trace:
        _nc_cache["last_result"] = res
    outp_t = np.zeros((B, NB, DIM // CI, P, CI), np.float32)
    attnT_t = np.zeros((B, NT, P, CI), np.float32)
    for r in res.results:
        outp_t += r["outp"]
        attnT_t += r["attnT"]
    outp = outp_t.transpose(0, 1, 3, 2, 4).reshape(B, N, DIM)
    outp += np.asarray(b_out, np.float32)
    attn_avg = np.zeros((B, N, N), np.float32)
    for t, (ci, jb) in enumerate(TILES):
        attn_avg[:, ci * CI:(ci + 1) * CI, jb * P:(jb + 1) * P] = \
            attnT_t[:, t].transpose(0, 2, 1)
    attn_avg /= np.float32(H)
    return np.ascontiguousarray(outp), attn_avg


# revision 24
# speedup vs baseline: 1.1384x; 1.0863x over previous
"""Trainium2 Bass kernel for nn_Attention_12189117186326 (sparse causal attention).

Sharding: tensor-parallel over heads — 16 heads / 8 cores = 2 heads per core,
both batch elements on every core.  Per-core partial outputs (head-slice of the
output projection, head-sum of the attention matrix) are combined on the host.

Per-core math (heads h0=2c, h0+1), matmuls in float32r (PE-rounded fp32):
  qT,kT  [64, n]   = Wq/Wk-slice^T @ x^T          (scores layout, q pre-scaled)
  vN     [n, 65]   = (x @ Wv-slice | ones)        (ones column -> softmax denom)
  sT     [128j, 512i] = kT-block^T . qT-chunk     (transposed scores, causal trapezoid only)
  sT    += I^T . biasT-tile                       (fp16 identity-matmul adds pos-bias
                                                   + causal -30000 on the PE, not DVE)
  p      = exp(sT + keymask_j)                    (keymask via ACT per-partition bias)
  oT|den [65, 512i]  += vN-block^T . p            (row 64 = softmax denominator)
  attnT  += p * (1/den)                           (1/den broadcast via PE outer product)
  outp   [n, 1024] += oT-block^T . Wout-slice

All large DMA streams use tile-major DRAM layouts (one contiguous burst per
tile); the host packs/unpacks.
"""
import numpy as np

B, N, DIM, H, DH = 2, 2048, 1024, 16, 64
INNER = H * DH
N_CORES = 8
HPC = 2              # heads per core
P = 128              # partitions / j-block
CI = 512             # i-chunk width (one PSUM bank of fp32)
NCI = N // CI        # 4 i-chunks
NB = N // P          # 16 j-blocks
NEG = -30000.0       # mask additive constant (exp underflows to exactly 0)
KT = DIM // P        # k-tiles in the projection contractions

# trapezoid tile enumeration: (ci, jb) for jb covering j <= i
TILES = [(ci, jb) for ci in range(NCI) for jb in range((ci + 1) * (CI // P))]
NT = len(TILES)      # 40
TILE_IDX = {t: n for n, t in enumerate(TILES)}

_nc_cache = {}


def _build():
    import concourse.tile as tile
    from concourse import bacc, mybir

    f32 = mybir.dt.float32
    f32r = mybir.dt.float32r
    f16 = mybir.dt.float16

    nc = bacc.Bacc("TRN2", target_bir_lowering=False, debug=False, num_devices=N_CORES)

    xT_d = nc.dram_tensor("xT", [B, NCI, P, KT, CI], f32r, kind="ExternalInput").ap()
    # wqk[:, 0, :] = [q_h0*scale | q_h1*scale], wqk[:, 1, :] = [k_h0 | k_h1]
    wqk_d = nc.dram_tensor("wqk", [DIM, 2, P], f32r, kind="ExternalInput").ap()
    wv_d = nc.dram_tensor("wv", [DIM, HPC * DH], f32r, kind="ExternalInput").ap()
    wout_d = nc.dram_tensor("wout", [HPC, DH, DIM], f32r, kind="ExternalInput").ap()
    eb_d = nc.dram_tensor("eb", [HPC, NT, P, CI], f16, kind="ExternalInput").ap()
    km_d = nc.dram_tensor("km", [B, NB, P], f32, kind="ExternalInput").ap()
    idf_d = nc.dram_tensor("idf", [P, P], f16, kind="ExternalInput").ap()
    onesr_d = nc.dram_tensor("onesr", [1, P], f32r, kind="ExternalInput").ap()
    onesv_d = nc.dram_tensor("onesv", [P, HPC, NB, 1], f32r, kind="ExternalInput").ap()
    outp_d = nc.dram_tensor("outp", [B, NB, DIM // CI, P, CI], f32, kind="ExternalOutput").ap()
    attnT_d = nc.dram_tensor("attnT", [B, NT, P, CI], f32r, kind="ExternalOutput").ap()

    from concourse.masks import make_identity

    with tile.TileContext(nc) as tc:
        with tc.tile_pool(name="w", bufs=1) as wpool:
            wqk_sb = wpool.tile([P, KT, 2, P], f32r, tag="wqk")
            nc.sync.dma_start(out=wqk_sb, in_=wqk_d.rearrange("(k p) q m -> p k q m", p=P))
            wv_sb = wpool.tile([P, KT, HPC * DH], f32r, tag="wv")
            nc.sync.dma_start(out=wv_sb, in_=wv_d.rearrange("(k p) m -> p k m", p=P))
            wout_sb = wpool.tile([DH, HPC, DIM], f32r, tag="wout")
            nc.sync.dma_start(out=wout_sb, in_=wout_d.rearrange("h p d -> p h d"))
            km_sb = wpool.tile([P, B, NB], f32, tag="km")
            nc.sync.dma_start(out=km_sb, in_=km_d.rearrange("b j p -> p b j"))
            ident = wpool.tile([P, P], f32, tag="ident")
            make_identity(nc, ident)
            idf_sb = wpool.tile([P, P], f16, tag="idf")
            nc.sync.dma_start(out=idf_sb, in_=idf_d)
            # ones row living on partition 64 — matmul lhsT/rhs bases must match
            # the denominator row of the attention PSUM (partition DH=64)
            ones_t = wpool.tile([DH + 1, P], f32r, tag="ones")
            nc.sync.dma_start(out=ones_t[DH:DH + 1, :], in_=onesr_d)

            for b in range(B):
                with tc.tile_pool(name="qkv", bufs=1) as qkvpool:
                    # heads stacked on partitions: hl0 -> 0..63, hl1 -> 64..127
                    qT_sb = qkvpool.tile([P, N], f32r, tag="qT")
                    kT_sb = qkvpool.tile([P, N], f32r, tag="kT")
                    vN_sb = qkvpool.tile([P, HPC, NB, DH + 1], f32r, tag="vN")
                    nc.sync.dma_start(out=vN_sb[:, :, :, DH:DH + 1], in_=onesv_d)
                    oT_sb = qkvpool.tile([DH, HPC, N], f32r, tag="oT")

                    # ---- phase 1: q/k/v projections for this batch ----
                    with tc.tile_pool(name="xt", bufs=2) as xpool, \
                         tc.tile_pool(name="psq", bufs=2, space="PSUM") as psq, \
                         tc.tile_pool(name="psv", bufs=2, space="PSUM") as psv, \
                         tc.tile_pool(name="pst", bufs=2, space="PSUM") as pstp, \
                         tc.tile_pool(name="vtmp", bufs=2) as vtpool:
                        for cc in range(NCI):
                            cs = cc * CI
                            xt = xpool.tile([P, KT, CI], f32r, tag="xt")
                            nc.sync.dma_start(out=xt, in_=xT_d[b, cc])
                            pq = psq.tile([P, CI], f32, tag="psq")
                            pk = psq.tile([P, CI], f32, tag="psk")
                            for k in range(KT):
                                for qk, ps in ((0, pq), (1, pk)):
                                    nc.tensor.matmul(
                                        out=ps[:],
                                        lhsT=wqk_sb[:, k, qk, :],
                                        rhs=xt[:, k, :],
                                        start=(k == 0), stop=(k == KT - 1))
                            nc.scalar.copy(out=qT_sb[:, cs:cs + CI], in_=pq[:])
                            nc.scalar.copy(out=kT_sb[:, cs:cs + CI], in_=pk[:])
                            # vT chunk [128(2h*64), 512] then transpose to natural
                            psvt = psv.tile([P, CI], f32, tag="psvt")
                            for k in range(KT):
                                nc.tensor.matmul(out=psvt[:], lhsT=wv_sb[:, k, :],
                                                 rhs=xt[:, k, :],
                                                 start=(k == 0), stop=(k == KT - 1))
                            vt = vtpool.tile([P, CI], f32, tag="vt")
                            nc.vector.tensor_copy(out=vt[:], in_=psvt[:])
                            for t in range(CI // P):
                                ptr = pstp.tile([P, P], f32, tag="ptr")
                                nc.tensor.transpose(ptr[:], vt[:, t * P:(t + 1) * P], ident[:])
                                jb = cc * (CI // P) + t
                                for hl in range(HPC):
                                    nc.vector.tensor_copy(
                                        out=vN_sb[:, hl, jb, 0:DH],
                                        in_=ptr[:, hl * DH:(hl + 1) * DH])

                    # ---- phase 2: attention + fused projection for this batch ----
                    # the two heads' pipelines are interleaved instruction-by-
                    # instruction; their K=64 score matmuls occupy disjoint PE
                    # row-groups (partitions 0-63 vs 64-127) and run concurrently
                    with tc.tile_pool(name="eb", bufs=8) as ebpool, \
                         tc.tile_pool(name="pp", bufs=36) as ppool, \
                         tc.tile_pool(name="rc", bufs=4) as rcpool, \
                         tc.tile_pool(name="ob", bufs=4) as obpool, \
                         tc.tile_pool(name="pssc", bufs=3, space="PSUM") as pssc, \
                         tc.tile_pool(name="psat", bufs=2, space="PSUM") as psat, \
                         tc.tile_pool(name="psrb", bufs=1, space="PSUM") as psrb, \
                         tc.tile_pool(name="pspr", bufs=2, space="PSUM") as pspr:
                        for ci in range(NCI):
                            cs = ci * CI
                            nj = (ci + 1) * (CI // P)
                            pa = [psat.tile([DH + 1, CI], f32, tag="pa", name="pa") for _ in range(HPC)]
                            pt = [[], []]
                            # software-pipelined: attnV for tile jb-1 is emitted
                            # between tile jb's scores and its exp, so the PE
                            # never waits on the ACT exp of the tile it just made
                            for jb in range(nj):
                                pscs = []
                                for hl in range(HPC):
                                    ps = pssc.tile([P, CI], f32, tag="pssc")
                                    nc.tensor.matmul(
                                        out=ps[:],
                                        lhsT=kT_sb[hl * DH:(hl + 1) * DH, jb * P:(jb + 1) * P],
                                        rhs=qT_sb[hl * DH:(hl + 1) * DH, cs:cs + CI],
                                        start=True, stop=False)
                                    pscs.append(ps)
                                for hl in range(HPC):
                                    ebt = ebpool.tile([P, CI], f16, tag="eb")
                                    nc.sync.dma_start(out=ebt, in_=eb_d[hl, TILE_IDX[(ci, jb)]])
                                    nc.tensor.matmul(
                                        out=pscs[hl][:], lhsT=idf_sb[:], rhs=ebt[:],
                                        start=False, stop=True)
                                if jb >= 1:
                                    for hl in range(HPC):
                                        nc.tensor.matmul(
                                            out=pa[hl][:], lhsT=vN_sb[:, hl, jb - 1, :],
                                            rhs=pt[hl][jb - 1][:],
                                            start=(jb == 1), stop=False)
                                for hl in range(HPC):
                                    p = ppool.tile([P, CI], f32r, tag="p")
                                    nc.scalar.activation(
                                        out=p[:], in_=pscs[hl][:],
                                        func=mybir.ActivationFunctionType.Exp,
                                        bias=km_sb[:, b, jb:jb + 1], scale=1.0)
                                    pt[hl].append(p)
                            for hl in range(HPC):
                                nc.tensor.matmul(
                                    out=pa[hl][:], lhsT=vN_sb[:, hl, nj - 1, :],
                                    rhs=pt[hl][nj - 1][:],
                                    start=(nj == 1), stop=True)
                            # denominator -> reciprocal (stays on partition 64) ->
                            # broadcast to all 128 partitions via PE outer product
                            rcss = []
                            for hl in range(HPC):
                                dn = rcpool.tile([DH + 1, CI], f32r, tag="dn")
                                with nc.allow_low_precision(reason="f32r is fp32-width"):
                                    nc.vector.reciprocal(dn[DH:DH + 1, :], pa[hl][DH:DH + 1, :])
                                rcb = psrb.tile([P, CI], f32, tag="rcb")
                                nc.tensor.matmul(out=rcb[:], lhsT=ones_t[DH:DH + 1, :],
                                                 rhs=dn[DH:DH + 1, :], start=True, stop=True)
                                rcs = rcpool.tile([P, CI], f32, tag="rcs")
                                nc.scalar.copy(out=rcs[:], in_=rcb[:])
                                rcss.append(rcs)
                                # normalized head output chunk
                                nc.vector.tensor_mul(
                                    oT_sb[:, hl, cs:cs + CI], pa[hl][0:DH, :], rcs[0:DH, :])
                            # normalize p tiles; accumulate heads; store attnT
                            for jb in range(nj):
                                nc.vector.tensor_mul(pt[0][jb][:], pt[0][jb][:], rcss[0][:])
                                nc.vector.tensor_mul(pt[1][jb][:], pt[1][jb][:], rcss[1][:])
                                nc.vector.tensor_add(pt[0][jb][:], pt[0][jb][:], pt[1][jb][:])
                                nc.sync.dma_start(out=attnT_d[b, TILE_IDX[(ci, jb)]],
                                                  in_=pt[0][jb][:])
                            # fused output projection for the n-blocks this ci completed
                            for nb in range(ci * (CI // P), (ci + 1) * (CI // P)):
                                for half in range(DIM // CI):
                                    ps = pspr.tile([P, CI], f32, tag="pspr")
                                    for hl in range(HPC):
                                        nc.tensor.matmul(
                                            out=ps[:],
                                            lhsT=oT_sb[:, hl, nb * P:(nb + 1) * P],
                                            rhs=wout_sb[:, hl, half * CI:(half + 1) * CI],
                                            start=(hl == 0), stop=(hl == HPC - 1))
                                    ob = obpool.tile([P, CI], f32, tag="ob")
                                    nc.scalar.copy(out=ob[:], in_=ps[:])
                                    nc.sync.dma_start(out=outp_d[b, nb, half], in_=ob[:])
    nc.compile()
    return nc


def _get_nc():
    if "nc" not in _nc_cache:
        _nc_cache["nc"] = _build()
    return _nc_cache["nc"]


def _prep_inputs(x, mask, positions_bias, W_qkv, W_out):
    scale = np.float32(DH ** -0.5)
    x = np.asarray(x, np.float32)
    mask = np.asarray(mask)
    pb = np.asarray(positions_bias, np.float32)
    W_qkv = np.asarray(W_qkv, np.float32)
    W_out = np.asarray(W_out, np.float32)

    # x tiled: xT_tiles[b, cc, p, k, n] = x[b, cc*CI+n, k*P+p]
    xT = np.ascontiguousarray(
        x.reshape(B, NCI, CI, KT, P).transpose(0, 1, 4, 3, 2))
    Wq, Wk, Wv = W_qkv[:, :INNER], W_qkv[:, INNER:2 * INNER], W_qkv[:, 2 * INNER:]
    km = np.where(mask, np.float32(NEG), np.float32(0.0)).astype(np.float32)
    km = np.ascontiguousarray(km.reshape(B, NB, P))

    # biasT[h, j, i] = pos_bias[h, i, j], NEG where j > i (causal); fp16, tile-major
    bT = pb[0].transpose(0, 2, 1)  # [H, j, i]
    tri = np.tri(N, dtype=bool).T  # [j, i], True where j <= i
    bT = np.where(tri[None, :, :], bT, np.float32(NEG)).astype(np.float16)
    eb_all = np.empty((H, NT, P, CI), np.float16)
    for t, (ci, jb) in enumerate(TILES):
        eb_all[:, t] = bT[:, jb * P:(jb + 1) * P, ci * CI:(ci + 1) * CI]

    in_maps = []
    for c in range(N_CORES):
        h0 = HPC * c
        wqk = np.empty((DIM, 2, P), np.float32)
        for hl in range(HPC):
            h = h0 + hl
            wqk[:, 0, hl * DH:(hl + 1) * DH] = Wq[:, h * DH:(h + 1) * DH] * scale
            wqk[:, 1, hl * DH:(hl + 1) * DH] = Wk[:, h * DH:(h + 1) * DH]
        wv = np.ascontiguousarray(
            np.concatenate([Wv[:, (h0 + hl) * DH:(h0 + hl + 1) * DH] for hl in range(HPC)], axis=1))
        wout = np.ascontiguousarray(
            np.stack([W_out[(h0 + hl) * DH:(h0 + hl + 1) * DH, :] for hl in range(HPC)]))
        in_maps.append({"xT": xT, "wqk": wqk, "wv": wv, "wout": wout,
                        "eb": np.ascontiguousarray(eb_all[h0:h0 + HPC]), "km": km,
                        "idf": np.eye(P, dtype=np.float16),
                        "onesr": np.ones((1, P), np.float32),
                        "onesv": np.ones((P, HPC, NB, 1), np.float32)})
    return in_maps


def kernel(x, mask, positions_bias, W_qkv, W_out, b_out, _trace=False):
    from concourse.bass_utils import run_bass_kernel_spmd
    nc = _get_nc()
    in_maps = _prep_inputs(x, mask, positions_bias, W_qkv, W_out)
    res = run_bass_kernel_spmd(nc, in_maps, list(range(N_CORES)), trace=_trace)
    if _trace:
        _nc_cache["last_result"] = res
    outp_t = np.zeros((B, NB, DIM // CI, P, CI), np.float32)
    attnT_t = np.zeros((B, NT, P, CI), np.float32)
    for r in res.results:
        outp_t += r["outp"]
        attnT_t += r["attnT"]
    outp = outp_t.transpose(0, 1, 3, 2, 4).reshape(B, N, DIM)
    outp += np.asarray(b_out, np.float32)
    attn_avg = np.zeros((B, N, N), np.float32)
    for t, (ci, jb) in enumerate(TILES):
        attn_avg[:, ci * CI:(ci + 1) * CI, jb * P:(jb + 1) * P] = \
            attnT_t[:, t].transpose(0, 2, 1)
    attn_avg /= np.float32(H)
    return np.ascontiguousarray(outp), attn_avg


# revision 27
# speedup vs baseline: 1.2747x; 1.1197x over previous
"""Trainium2 Bass kernel for nn_Attention_12189117186326 (sparse causal attention).

Sharding: tensor-parallel over heads — 16 heads / 8 cores = 2 heads per core,
both batch elements on every core.  Per-core partial outputs (head-slice of the
output projection, head-sum of the attention matrix) are combined on the host.

Per-core math (heads h0=2c, h0+1), matmuls in float32r (PE-rounded fp32):
  qT,kT  [64, n]   = Wq/Wk-slice^T @ x^T          (scores layout, q pre-scaled)
  vN     [n, 65]   = (x @ Wv-slice | ones)        (ones column -> softmax denom)
  sT     [128j, 512i] = kT-block^T . qT-chunk     (transposed scores, causal trapezoid only)
  sT    += I^T . biasT-tile                       (fp16 identity-matmul adds pos-bias
                                                   + causal -30000 on the PE, not DVE)
  p      = exp(sT + keymask_j)                    (keymask via ACT per-partition bias)
  oT|den [65, 512i]  += vN-block^T . p            (row 64 = softmax denominator)
  attnT  += p * (1/den)                           (1/den broadcast via PE outer product)
  outp   [n, 1024] += oT-block^T . Wout-slice

All large DMA streams use tile-major DRAM layouts (one contiguous burst per
tile); the host packs/unpacks.
"""
import numpy as np

B, N, DIM, H, DH = 2, 2048, 1024, 16, 64
INNER = H * DH
N_CORES = 8
HPC = 2              # heads per core
P = 128              # partitions / j-block
CI = 512             # i-chunk width (one PSUM bank of fp32)
NCI = N // CI        # 4 i-chunks
NB = N // P          # 16 j-blocks
NEG = -30000.0       # mask additive constant (exp underflows to exactly 0)
KT = DIM // P        # k-tiles in the projection contractions

# trapezoid tile enumeration: (ci, jb) for jb covering j <= i
TILES = [(ci, jb) for ci in range(NCI) for jb in range((ci + 1) * (CI // P))]
NT = len(TILES)      # 40
TILE_IDX = {t: n for n, t in enumerate(TILES)}

_nc_cache = {}


def _build():
    import concourse.tile as tile
    from concourse import bacc, mybir

    f32 = mybir.dt.float32
    f32r = mybir.dt.float32r
    f16 = mybir.dt.float16

    nc = bacc.Bacc("TRN2", target_bir_lowering=False, debug=False, num_devices=N_CORES)

    xT_d = nc.dram_tensor("xT", [B, NCI, P, KT, CI], f32r, kind="ExternalInput").ap()
    # wqk[:, 0, :] = [q_h0*scale | q_h1*scale], wqk[:, 1, :] = [k_h0 | k_h1]
    wqk_d = nc.dram_tensor("wqk", [DIM, 2, P], f32r, kind="ExternalInput").ap()
    wv_d = nc.dram_tensor("wv", [DIM, HPC * DH], f32r, kind="ExternalInput").ap()
    wout_d = nc.dram_tensor("wout", [HPC, DH, DIM], f32r, kind="ExternalInput").ap()
    eb_d = nc.dram_tensor("eb", [HPC, NT, P, CI], f16, kind="ExternalInput").ap()
    km_d = nc.dram_tensor("km", [B, NB, P], f32, kind="ExternalInput").ap()
    idf_d = nc.dram_tensor("idf", [P, P], f16, kind="ExternalInput").ap()
    onesr_d = nc.dram_tensor("onesr", [1, P], f32r, kind="ExternalInput").ap()
    onesv_d = nc.dram_tensor("onesv", [P, HPC, NB, 1], f32r, kind="ExternalInput").ap()
    outp_d = nc.dram_tensor("outp", [B, NB, DIM // CI, P, CI], f32, kind="ExternalOutput").ap()
    attnT_d = nc.dram_tensor("attnT", [B, NT, P, CI], f32r, kind="ExternalOutput").ap()

    from concourse.masks import make_identity

    with tile.TileContext(nc) as tc:
        with tc.tile_pool(name="w", bufs=1) as wpool:
            wqk_sb = wpool.tile([P, KT, 2, P], f32r, tag="wqk")
            nc.sync.dma_start(out=wqk_sb, in_=wqk_d.rearrange("(k p) q m -> p k q m", p=P))
            wv_sb = wpool.tile([P, KT, HPC * DH], f32r, tag="wv")
            nc.sync.dma_start(out=wv_sb, in_=wv_d.rearrange("(k p) m -> p k m", p=P))
            wout_sb = wpool.tile([DH, HPC, DIM], f32r, tag="wout")
            nc.sync.dma_start(out=wout_sb, in_=wout_d.rearrange("h p d -> p h d"))
            km_sb = wpool.tile([P, B, NB], f32, tag="km")
            nc.sync.dma_start(out=km_sb, in_=km_d.rearrange("b j p -> p b j"))
            ident = wpool.tile([P, P], f32, tag="ident")
            make_identity(nc, ident)
            idf_sb = wpool.tile([P, P], f16, tag="idf")
            nc.sync.dma_start(out=idf_sb, in_=idf_d)
            # ones row living on partition 64 — matmul lhsT/rhs bases must match
            # the denominator row of the attention PSUM (partition DH=64)
            ones_t = wpool.tile([DH + 1, P], f32r, tag="ones")
            nc.sync.dma_start(out=ones_t[DH:DH + 1, :], in_=onesr_d)

            from contextlib import ExitStack
            _gctx = ExitStack()
            xpool = _gctx.enter_context(tc.tile_pool(name="xt", bufs=2))
            ebpool = _gctx.enter_context(tc.tile_pool(name="eb", bufs=8))
            ppool = _gctx.enter_context(tc.tile_pool(name="pp", bufs=34))
            rcpool = _gctx.enter_context(tc.tile_pool(name="rc", bufs=2))
            obpool = _gctx.enter_context(tc.tile_pool(name="ob", bufs=4))
            vtpool = _gctx.enter_context(tc.tile_pool(name="vtmp", bufs=2))

            for b in range(B):
                with tc.tile_pool(name="qkv", bufs=1) as qkvpool:
                    # heads stacked on partitions: hl0 -> 0..63, hl1 -> 64..127
                    qT_sb = qkvpool.tile([P, N], f32r, tag="qT")
                    kT_sb = qkvpool.tile([P, N], f32r, tag="kT")
                    vN_sb = qkvpool.tile([P, HPC, NB, DH + 1], f32r, tag="vN")
                    nc.sync.dma_start(out=vN_sb[:, :, :, DH:DH + 1], in_=onesv_d)
                    oT_sb = qkvpool.tile([DH, HPC, N], f32r, tag="oT")

                    # ---- phase 1: q/k/v projections for this batch ----
                    with tc.tile_pool(name="psq", bufs=2, space="PSUM") as psq, \
                         tc.tile_pool(name="psv", bufs=2, space="PSUM") as psv, \
                         tc.tile_pool(name="pst", bufs=2, space="PSUM") as pstp:
                        for cc in range(NCI):
                            cs = cc * CI
                            xt = xpool.tile([P, KT, CI], f32r, tag="xt")
                            nc.sync.dma_start(out=xt, in_=xT_d[b, cc])
                            pq = psq.tile([P, CI], f32, tag="psq")
                            pk = psq.tile([P, CI], f32, tag="psk")
                            for k in range(KT):
                                for qk, ps in ((0, pq), (1, pk)):
                                    nc.tensor.matmul(
                                        out=ps[:],
                                        lhsT=wqk_sb[:, k, qk, :],
                                        rhs=xt[:, k, :],
                                        start=(k == 0), stop=(k == KT - 1))
                            nc.scalar.copy(out=qT_sb[:, cs:cs + CI], in_=pq[:])
                            nc.scalar.copy(out=kT_sb[:, cs:cs + CI], in_=pk[:])
                            # vT chunk [128(2h*64), 512] then transpose to natural
                            psvt = psv.tile([P, CI], f32, tag="psvt")
                            for k in range(KT):
                                nc.tensor.matmul(out=psvt[:], lhsT=wv_sb[:, k, :],
                                                 rhs=xt[:, k, :],
                                                 start=(k == 0), stop=(k == KT - 1))
                            vt = vtpool.tile([P, CI], f32, tag="vt")
                            nc.vector.tensor_copy(out=vt[:], in_=psvt[:])
                            for t in range(CI // P):
                                ptr = pstp.tile([P, P], f32, tag="ptr")
                                nc.tensor.transpose(ptr[:], vt[:, t * P:(t + 1) * P], ident[:])
                                jb = cc * (CI // P) + t
                                for hl in range(HPC):
                                    nc.vector.tensor_copy(
                                        out=vN_sb[:, hl, jb, 0:DH],
                                        in_=ptr[:, hl * DH:(hl + 1) * DH])

                    # ---- phase 2: attention + fused projection for this batch ----
                    # the two heads' pipelines are interleaved instruction-by-
                    # instruction; their K=64 score matmuls occupy disjoint PE
                    # row-groups (partitions 0-63 vs 64-127) and run concurrently
                    with tc.tile_pool(name="pssc", bufs=3, space="PSUM") as pssc, \
                         tc.tile_pool(name="psat", bufs=2, space="PSUM") as psat, \
                         tc.tile_pool(name="psrb", bufs=1, space="PSUM") as psrb, \
                         tc.tile_pool(name="pspr", bufs=2, space="PSUM") as pspr:
                        for ci in reversed(range(NCI)):
                            cs = ci * CI
                            nj = (ci + 1) * (CI // P)
                            pa = [psat.tile([DH + 1, CI], f32, tag="pa", name="pa") for _ in range(HPC)]
                            pt = [[], []]
                            # software-pipelined: attnV for tile jb-1 is emitted
                            # between tile jb's scores and its exp, so the PE
                            # never waits on the ACT exp of the tile it just made
                            for jb in range(nj):
                                pscs = []
                                for hl in range(HPC):
                                    ps = pssc.tile([P, CI], f32, tag="pssc")
                                    nc.tensor.matmul(
                                        out=ps[:],
                                        lhsT=kT_sb[hl * DH:(hl + 1) * DH, jb * P:(jb + 1) * P],
                                        rhs=qT_sb[hl * DH:(hl + 1) * DH, cs:cs + CI],
                                        start=True, stop=False)
                                    pscs.append(ps)
                                for hl in range(HPC):
                                    ebt = ebpool.tile([P, CI], f16, tag="eb")
                                    nc.sync.dma_start(out=ebt, in_=eb_d[hl, TILE_IDX[(ci, jb)]])
                                    nc.tensor.matmul(
                                        out=pscs[hl][:], lhsT=idf_sb[:], rhs=ebt[:],
                                        start=False, stop=True)
                                if jb >= 1:
                                    for hl in range(HPC):
                                        nc.tensor.matmul(
                                            out=pa[hl][:], lhsT=vN_sb[:, hl, jb - 1, :],
                                            rhs=pt[hl][jb - 1][:],
                                            start=(jb == 1), stop=False)
                                for hl in range(HPC):
                                    p = ppool.tile([P, CI], f32r, tag="p")
                                    nc.scalar.activation(
                                        out=p[:], in_=pscs[hl][:],
                                        func=mybir.ActivationFunctionType.Exp,
                                        bias=km_sb[:, b, jb:jb + 1], scale=1.0)
                                    pt[hl].append(p)
                            for hl in range(HPC):
                                nc.tensor.matmul(
                                    out=pa[hl][:], lhsT=vN_sb[:, hl, nj - 1, :],
                                    rhs=pt[hl][nj - 1][:],
                                    start=(nj == 1), stop=True)
                            # denominator -> reciprocal (stays on partition 64) ->
                            # broadcast to all 128 partitions via PE outer product
                            rcss = []
                            for hl in range(HPC):
                                dn = rcpool.tile([DH + 1, CI], f32r, tag="dn")
                                with nc.allow_low_precision(reason="f32r is fp32-width"):
                                    nc.vector.reciprocal(dn[DH:DH + 1, :], pa[hl][DH:DH + 1, :])
                                rcb = psrb.tile([P, CI], f32, tag="rcb")
                                nc.tensor.matmul(out=rcb[:], lhsT=ones_t[DH:DH + 1, :],
                                                 rhs=dn[DH:DH + 1, :], start=True, stop=True)
                                rcs = rcpool.tile([P, CI], f32, tag="rcs")
                                nc.scalar.copy(out=rcs[:], in_=rcb[:])
                                rcss.append(rcs)
                                # normalized head output chunk
                                nc.vector.tensor_mul(
                                    oT_sb[:, hl, cs:cs + CI], pa[hl][0:DH, :], rcs[0:DH, :])
                            # normalize p tiles; accumulate heads; store attnT
                            for jb in range(nj):
                                nc.vector.tensor_mul(pt[0][jb][:], pt[0][jb][:], rcss[0][:])
                                nc.vector.tensor_mul(pt[1][jb][:], pt[1][jb][:], rcss[1][:])
                                nc.vector.tensor_add(pt[0][jb][:], pt[0][jb][:], pt[1][jb][:])
                                nc.gpsimd.dma_start(out=attnT_d[b, TILE_IDX[(ci, jb)]],
                                                  in_=pt[0][jb][:])
                            # fused output projection for the n-blocks this ci completed
                            for nb in range(ci * (CI // P), (ci + 1) * (CI // P)):
                                for half in range(DIM // CI):
                                    ps = pspr.tile([P, CI], f32, tag="pspr")
                                    for hl in range(HPC):
                                        nc.tensor.matmul(
                                            out=ps[:],
                                            lhsT=oT_sb[:, hl, nb * P:(nb + 1) * P],
                                            rhs=wout_sb[:, hl, half * CI:(half + 1) * CI],
                                            start=(hl == 0), stop=(hl == HPC - 1))
                                    ob = obpool.tile([P, CI], f32, tag="ob")
                                    nc.scalar.copy(out=ob[:], in_=ps[:])
                                    nc.gpsimd.dma_start(out=outp_d[b, nb, half], in_=ob[:])
            _gctx.close()
    nc.compile()
    return nc


def _get_nc():
    if "nc" not in _nc_cache:
        _nc_cache["nc"] = _build()
    return _nc_cache["nc"]


def _prep_inputs(x, mask, positions_bias, W_qkv, W_out):
    scale = np.float32(DH ** -0.5)
    x = np.asarray(x, np.float32)
    mask = np.asarray(mask)
    pb = np.asarray(positions_bias, np.float32)
    W_qkv = np.asarray(W_qkv, np.float32)
    W_out = np.asarray(W_out, np.float32)

    # x tiled: xT_tiles[b, cc, p, k, n] = x[b, cc*CI+n, k*P+p]
    xT = np.ascontiguousarray(
        x.reshape(B, NCI, CI, KT, P).transpose(0, 1, 4, 3, 2))
    Wq, Wk, Wv = W_qkv[:, :INNER], W_qkv[:, INNER:2 * INNER], W_qkv[:, 2 * INNER:]
    km = np.where(mask, np.float32(NEG), np.float32(0.0)).astype(np.float32)
    km = np.ascontiguousarray(km.reshape(B, NB, P))

    # biasT[h, j, i] = pos_bias[h, i, j], NEG where j > i (causal); fp16, tile-major
    bT = pb[0].transpose(0, 2, 1)  # [H, j, i]
    tri = np.tri(N, dtype=bool).T  # [j, i], True where j <= i
    bT = np.where(tri[None, :, :], bT, np.float32(NEG)).astype(np.float16)
    eb_all = np.empty((H, NT, P, CI), np.float16)
    for t, (ci, jb) in enumerate(TILES):
        eb_all[:, t] = bT[:, jb * P:(jb + 1) * P, ci * CI:(ci + 1) * CI]

    in_maps = []
    for c in range(N_CORES):
        h0 = HPC * c
        wqk = np.empty((DIM, 2, P), np.float32)
        for hl in range(HPC):
            h = h0 + hl
            wqk[:, 0, hl * DH:(hl + 1) * DH] = Wq[:, h * DH:(h + 1) * DH] * scale
            wqk[:, 1, hl * DH:(hl + 1) * DH] = Wk[:, h * DH:(h + 1) * DH]
        wv = np.ascontiguousarray(
            np.concatenate([Wv[:, (h0 + hl) * DH:(h0 + hl + 1) * DH] for hl in range(HPC)], axis=1))
        wout = np.ascontiguousarray(
            np.stack([W_out[(h0 + hl) * DH:(h0 + hl + 1) * DH, :] for hl in range(HPC)]))
        in_maps.append({"xT": xT, "wqk": wqk, "wv": wv, "wout": wout,
                        "eb": np.ascontiguousarray(eb_all[h0:h0 + HPC]), "km": km,
                        "idf": np.eye(P, dtype=np.float16),
                        "onesr": np.ones((1, P), np.float32),
                        "onesv": np.ones((P, HPC, NB, 1), np.float32)})
    return in_maps


def kernel(x, mask, positions_bias, W_qkv, W_out, b_out, _trace=False):
    from concourse.bass_utils import run_bass_kernel_spmd
    nc = _get_nc()
    in_maps = _prep_inputs(x, mask, positions_bias, W_qkv, W_out)
    res = run_bass_kernel_spmd(nc, in_maps, list(range(N_CORES)), trace=_trace)
    if _trace:
        _nc_cache["last_result"] = res
    outp_t = np.zeros((B, NB, DIM // CI, P, CI), np.float32)
    attnT_t = np.zeros((B, NT, P, CI), np.float32)
    for r in res.results:
        outp_t += r["outp"]
        attnT_t += r["attnT"]
    outp = outp_t.transpose(0, 1, 3, 2, 4).reshape(B, N, DIM)
    outp += np.asarray(b_out, np.float32)
    attn_avg = np.zeros((B, N, N), np.float32)
    for t, (ci, jb) in enumerate(TILES):
        attn_avg[:, ci * CI:(ci + 1) * CI, jb * P:(jb + 1) * P] = \
            attnT_t[:, t].transpose(0, 2, 1)
    attn_avg /= np.float32(H)
    return np.ascontiguousarray(outp), attn_avg


# revision 29
# speedup vs baseline: 1.3780x; 1.0810x over previous
"""Trainium2 Bass kernel for nn_Attention_12189117186326 (sparse causal attention).

Sharding: tensor-parallel over heads — 16 heads / 8 cores = 2 heads per core,
both batch elements on every core.  Per-core partial outputs (head-slice of the
output projection, head-sum of the attention matrix) are combined on the host.

Per-core math (heads h0=2c, h0+1), matmuls in float32r (PE-rounded fp32):
  qT,kT  [128, n]  = Wq/Wk-2-head-slice^T @ x^T   (heads stacked on partitions,
                                                   q pre-scaled by 1/sqrt(dh))
  sT     [128j, 512i] = kT-rows^T . qT-rows       (transposed scores, causal
                                                   trapezoid tiles only)
  sT    += I^T . biasT-tile                       (fp16 identity-matmul adds pos-bias
                                                   + causal -30000 on the PE, not DVE)
  p      = exp(sT + keymask_j)                    (keymask via ACT per-partition bias)
  pa     [*, 512i]   += vN-block^T . p            (vN carries a ones column so one
                                                   PSUM row is the softmax denominator;
                                                   head 1's v lands on partitions 64-127
                                                   so oT ends up head-stacked)
  attnT  += p * (1/den)                           (1/den broadcast via PE outer product;
                                                   written to HBM as fp16 tiles)
  outp   [n, 1024] = oT-2-head-stack^T . Wout     (single K=128 matmul per tile)

All large DMA streams use tile-major DRAM layouts (one contiguous burst per
tile); the host packs/unpacks.
"""
import numpy as np

B, N, DIM, H, DH = 2, 2048, 1024, 16, 64
INNER = H * DH
N_CORES = 8
HPC = 2              # heads per core
P = 128              # partitions / j-block
CI = 512             # i-chunk width (one PSUM bank of fp32)
NCI = N // CI        # 4 i-chunks
NB = N // P          # 16 j-blocks
NEG = -30000.0       # mask additive constant (exp underflows to exactly 0)
KT = DIM // P        # k-tiles in the projection contractions

# trapezoid tile enumeration: (ci, jb) for jb covering j <= i
TILES = [(ci, jb) for ci in range(NCI) for jb in range((ci + 1) * (CI // P))]
NT = len(TILES)      # 40
TILE_IDX = {t: n for n, t in enumerate(TILES)}

_nc_cache = {}


def _build():
    import concourse.tile as tile
    from concourse import bacc, mybir
    from contextlib import ExitStack

    f32 = mybir.dt.float32
    f32r = mybir.dt.float32r
    f16 = mybir.dt.float16

    nc = bacc.Bacc("TRN2", target_bir_lowering=False, debug=False, num_devices=N_CORES)

    xT_d = nc.dram_tensor("xT", [B, NCI, KT, P, CI], f32r, kind="ExternalInput").ap()
    # wqk[:, 0, :] = [q_h0*scale | q_h1*scale], wqk[:, 1, :] = [k_h0 | k_h1]
    wqk_d = nc.dram_tensor("wqk", [DIM, 2, P], f32r, kind="ExternalInput").ap()
    wv_d = nc.dram_tensor("wv", [DIM, HPC * DH], f32r, kind="ExternalInput").ap()
    wout_d = nc.dram_tensor("wout", [P, DIM], f32r, kind="ExternalInput").ap()
    eb_d = nc.dram_tensor("eb", [HPC, NT, P, CI], f16, kind="ExternalInput").ap()
    km_d = nc.dram_tensor("km", [B, NB, P], f32, kind="ExternalInput").ap()
    idf_d = nc.dram_tensor("idf", [P, P], f16, kind="ExternalInput").ap()
    onesr_d = nc.dram_tensor("onesr", [1, P], f32r, kind="ExternalInput").ap()
    # vN skeleton: ones/zero columns per head (v columns overwritten on device)
    vinit_d = nc.dram_tensor("vinit", [P, HPC, P], f32r, kind="ExternalInput").ap()
    outp_d = nc.dram_tensor("outp", [B, NB, DIM // CI, P, CI], f32, kind="ExternalOutput").ap()
    attnT_d = nc.dram_tensor("attnT", [B, NT, P, CI], f16, kind="ExternalOutput").ap()

    from concourse.masks import make_identity

    # per-head vN column layout: head 0 -> v at cols 0..63, ones col 64 (denom row 64)
    #                            head 1 -> ones col 0 (denom row 0), v at cols 64..127
    VCOL = (0, DH)       # v column offset per head
    NCOL = (DH, 0)       # ones column per head
    DROW = (DH, 0)       # denominator PSUM row per head
    OROW = (0, DH)       # oT PSUM row base per head
    MWID = (DH + 1, P)   # lhsT width per head

    with tile.TileContext(nc) as tc:
        with tc.tile_pool(name="w", bufs=1) as wpool:
            wqk_sb = wpool.tile([P, KT, 2, P], f32r, tag="wqk")
            nc.sync.dma_start(out=wqk_sb, in_=wqk_d.rearrange("(k p) q m -> p k q m", p=P))
            wv_sb = wpool.tile([P, KT, HPC * DH], f32r, tag="wv")
            nc.sync.dma_start(out=wv_sb, in_=wv_d.rearrange("(k p) m -> p k m", p=P))
            wout_sb = wpool.tile([P, DIM], f32r, tag="wout")
            nc.sync.dma_start(out=wout_sb, in_=wout_d)
            km_sb = wpool.tile([P, B, NB], f32, tag="km")
            nc.sync.dma_start(out=km_sb, in_=km_d.rearrange("b j p -> p b j"))
            ident = wpool.tile([P, P], f32, tag="ident")
            make_identity(nc, ident)
            idf_sb = wpool.tile([P, P], f16, tag="idf")
            nc.sync.dma_start(out=idf_sb, in_=idf_d)
            # ones rows on partition 64 (head 0 denom) and partition 0 (head 1 denom)
            ones_t = wpool.tile([DH + 1, P], f32r, tag="ones")
            nc.sync.dma_start(out=ones_t[DH:DH + 1, :], in_=onesr_d)
            ones_z = wpool.tile([1, P], f32r, tag="onesz")
            nc.sync.dma_start(out=ones_z, in_=onesr_d)
            ones_row = (ones_t[DH:DH + 1, :], ones_z[:])

            _gctx = ExitStack()
            xpool = _gctx.enter_context(tc.tile_pool(name="xt", bufs=2 * KT))
            ebpool = _gctx.enter_context(tc.tile_pool(name="eb", bufs=10))
            ppool = _gctx.enter_context(tc.tile_pool(name="pp", bufs=34))
            rcpool = _gctx.enter_context(tc.tile_pool(name="rc", bufs=2))
            obpool = _gctx.enter_context(tc.tile_pool(name="ob", bufs=4))
            vtpool = _gctx.enter_context(tc.tile_pool(name="vtmp", bufs=2))

            for b in range(B):
                with tc.tile_pool(name="qkv", bufs=1) as qkvpool:
                    # heads stacked on partitions: hl0 -> 0..63, hl1 -> 64..127
                    qT_sb = qkvpool.tile([P, N], f32r, tag="qT")
                    kT_sb = qkvpool.tile([P, N], f32r, tag="kT")
                    vN_sb = qkvpool.tile([P, HPC, NB, P], f32r, tag="vN")
                    nc.sync.dma_start(
                        out=vN_sb,
                        in_=vinit_d.rearrange("p h (o m) -> p h o m", o=1)
                        .broadcast_to([P, HPC, NB, P]))
                    oT_sb = qkvpool.tile([P, N], f32r, tag="oT")

                    # ---- phase 1: q/k/v projections for this batch ----
                    with tc.tile_pool(name="psq", bufs=2, space="PSUM") as psq, \
                         tc.tile_pool(name="psv", bufs=2, space="PSUM") as psv, \
                         tc.tile_pool(name="pst", bufs=2, space="PSUM") as pstp:
                        for cc in range(NCI):
                            cs = cc * CI
                            xts = []
                            for k in range(KT):
                                xt = xpool.tile([P, CI], f32r, tag="xt", name="xt")
                                nc.sync.dma_start(out=xt, in_=xT_d[b, cc, k])
                                xts.append(xt)
                            pq = psq.tile([P, CI], f32, tag="psq")
                            pk = psq.tile([P, CI], f32, tag="psk")
                            for k in range(KT):
                                for qk, ps in ((0, pq), (1, pk)):
                                    nc.tensor.matmul(
                                        out=ps[:],
                                        lhsT=wqk_sb[:, k, qk, :],
                                        rhs=xts[k][:],
                                        start=(k == 0), stop=(k == KT - 1))
                            nc.scalar.copy(out=qT_sb[:, cs:cs + CI], in_=pq[:])
                            nc.scalar.copy(out=kT_sb[:, cs:cs + CI], in_=pk[:])
                            # vT chunk [128(2h*64), 512] then transpose to natural
                            psvt = psv.tile([P, CI], f32, tag="psvt")
                            for k in range(KT):
                                nc.tensor.matmul(out=psvt[:], lhsT=wv_sb[:, k, :],
                                                 rhs=xts[k][:],
                                                 start=(k == 0), stop=(k == KT - 1))
                            vt = vtpool.tile([P, CI], f32, tag="vt")
                            nc.vector.tensor_copy(out=vt[:], in_=psvt[:])
                            for t in range(CI // P):
                                ptr = pstp.tile([P, P], f32, tag="ptr")
                                nc.tensor.transpose(ptr[:], vt[:, t * P:(t + 1) * P], ident[:])
                                jb = cc * (CI // P) + t
                                for hl in range(HPC):
                                    nc.vector.tensor_copy(
                                        out=vN_sb[:, hl, jb, VCOL[hl]:VCOL[hl] + DH],
                                        in_=ptr[:, hl * DH:(hl + 1) * DH])

                    # ---- phase 2: attention + fused projection for this batch ----
                    # the two heads' pipelines are interleaved instruction-by-
                    # instruction to keep every engine fed
                    with tc.tile_pool(name="pssc", bufs=3, space="PSUM") as pssc, \
                         tc.tile_pool(name="psat", bufs=2, space="PSUM") as psat, \
                         tc.tile_pool(name="psrb", bufs=1, space="PSUM") as psrb, \
                         tc.tile_pool(name="pspr", bufs=2, space="PSUM") as pspr:
                        for ci in reversed(range(NCI)):
                            cs = ci * CI
                            nj = (ci + 1) * (CI // P)
                            pa = [psat.tile([P, CI], f32, tag="pa", name="pa")
                                  for _ in range(HPC)]
                            pt = [[], []]
                            # software-pipelined: attnV for tile jb-1 is emitted
                            # between tile jb's scores and its exp, so the PE
                            # never waits on the ACT exp of the tile it just made
                            for jb in range(nj):
                                pscs = []
                                for hl in range(HPC):
                                    ps = pssc.tile([P, CI], f32, tag="pssc", name="pssc")
                                    nc.tensor.matmul(
                                        out=ps[:],
                                        lhsT=kT_sb[hl * DH:(hl + 1) * DH, jb * P:(jb + 1) * P],
                                        rhs=qT_sb[hl * DH:(hl + 1) * DH, cs:cs + CI],
                                        start=True, stop=False)
                                    pscs.append(ps)
                                for hl in range(HPC):
                                    ebt = ebpool.tile([P, CI], f16, tag="eb", name="eb")
                                    nc.sync.dma_start(out=ebt, in_=eb_d[hl, TILE_IDX[(ci, jb)]])
                                    nc.tensor.matmul(
                                        out=pscs[hl][:], lhsT=idf_sb[:], rhs=ebt[:],
                                        start=False, stop=True)
                                if jb >= 1:
                                    for hl in range(HPC):
                                        nc.tensor.matmul(
                                            out=pa[hl][0:MWID[hl], :],
                                            lhsT=vN_sb[:, hl, jb - 1, :MWID[hl]],
                                            rhs=pt[hl][jb - 1][:],
                                            start=(jb == 1), stop=False)
                                for hl in range(HPC):
                                    p = ppool.tile([P, CI], f32r, tag="p", name="p")
                                    nc.scalar.activation(
                                        out=p[:], in_=pscs[hl][:],
                                        func=mybir.ActivationFunctionType.Exp,
                                        bias=km_sb[:, b, jb:jb + 1], scale=1.0)
                                    pt[hl].append(p)
                            for hl in range(HPC):
                                nc.tensor.matmul(
                                    out=pa[hl][0:MWID[hl], :],
                                    lhsT=vN_sb[:, hl, nj - 1, :MWID[hl]],
                                    rhs=pt[hl][nj - 1][:],
                                    start=(nj == 1), stop=True)
                            # denominator -> reciprocal (on its head's PSUM row) ->
                            # broadcast to all 128 partitions via PE outer product
                            rcss = []
                            for hl in range(HPC):
                                dr = DROW[hl]
                                dn = rcpool.tile([DH + 1, CI], f32r, tag="dn", name="dn")
                                with nc.allow_low_precision(reason="f32r is fp32-width"):
                                    nc.vector.reciprocal(dn[dr:dr + 1, :], pa[hl][dr:dr + 1, :])
                                rcb = psrb.tile([P, CI], f32, tag="rcb")
                                nc.tensor.matmul(out=rcb[:], lhsT=ones_row[hl],
                                                 rhs=dn[dr:dr + 1, :], start=True, stop=True)
                                rcs = rcpool.tile([P, CI], f32, tag="rcs", name="rcs")
                                nc.scalar.copy(out=rcs[:], in_=rcb[:])
                                rcss.append(rcs)
                                # normalized head output chunk (head-stacked oT rows)
                                orow = OROW[hl]
                                nc.vector.tensor_mul(
                                    oT_sb[orow:orow + DH, cs:cs + CI],
                                    pa[hl][orow:orow + DH, :], rcs[orow:orow + DH, :])
                            # normalize p tiles; accumulate heads; store attnT (fp16)
                            for jb in range(nj):
                                nc.vector.tensor_mul(pt[0][jb][:], pt[0][jb][:], rcss[0][:])
                                nc.vector.tensor_mul(pt[1][jb][:], pt[1][jb][:], rcss[1][:])
                                nc.vector.tensor_add(pt[0][jb][:], pt[0][jb][:], pt[1][jb][:])
                                nc.gpsimd.dma_start(out=attnT_d[b, TILE_IDX[(ci, jb)]],
                                                    in_=pt[0][jb][:].bitcast(f32))
                            # fused output projection (single K=128 matmul per tile)
                            for nb in range(ci * (CI // P), (ci + 1) * (CI // P)):
                                for half in range(DIM // CI):
                                    ps = pspr.tile([P, CI], f32, tag="pspr")
                                    nc.tensor.matmul(
                                        out=ps[:],
                                        lhsT=oT_sb[:, nb * P:(nb + 1) * P],
                                        rhs=wout_sb[:, half * CI:(half + 1) * CI],
                                        start=True, stop=True)
                                    ob = obpool.tile([P, CI], f32, tag="ob")
                                    nc.scalar.copy(out=ob[:], in_=ps[:])
                                    nc.gpsimd.dma_start(out=outp_d[b, nb, half], in_=ob[:])
            _gctx.close()
    nc.compile()
    return nc


def _get_nc():
    if "nc" not in _nc_cache:
        _nc_cache["nc"] = _build()
    return _nc_cache["nc"]


def _prep_inputs(x, mask, positions_bias, W_qkv, W_out):
    scale = np.float32(DH ** -0.5)
    x = np.asarray(x, np.float32)
    mask = np.asarray(mask)
    pb = np.asarray(positions_bias, np.float32)
    W_qkv = np.asarray(W_qkv, np.float32)
    W_out = np.asarray(W_out, np.float32)

    # x tiled: xT_tiles[b, cc, k, p, n] = x[b, cc*CI+n, k*P+p]
    xT = np.ascontiguousarray(
        x.reshape(B, NCI, CI, KT, P).transpose(0, 1, 3, 4, 2))
    Wq, Wk, Wv = W_qkv[:, :INNER], W_qkv[:, INNER:2 * INNER], W_qkv[:, 2 * INNER:]
    km = np.where(mask, np.float32(NEG), np.float32(0.0)).astype(np.float32)
    km = np.ascontiguousarray(km.reshape(B, NB, P))

    # biasT[h, j, i] = pos_bias[h, i, j], NEG where j > i (causal); fp16, tile-major
    bT = pb[0].transpose(0, 2, 1)  # [H, j, i]
    tri = np.tri(N, dtype=bool).T  # [j, i], True where j <= i
    bT = np.where(tri[None, :, :], bT, np.float32(NEG)).astype(np.float16)
    eb_all = np.empty((H, NT, P, CI), np.float16)
    for t, (ci, jb) in enumerate(TILES):
        eb_all[:, t] = bT[:, jb * P:(jb + 1) * P, ci * CI:(ci + 1) * CI]

    vinit = np.zeros((P, HPC, P), np.float32)
    vinit[:, 0, DH] = 1.0   # head 0: ones col 64 -> denom row 64
    vinit[:, 1, 0] = 1.0    # head 1: ones col 0  -> denom row 0

    in_maps = []
    for c in range(N_CORES):
        h0 = HPC * c
        wqk = np.empty((DIM, 2, P), np.float32)
        for hl in range(HPC):
            h = h0 + hl
            wqk[:, 0, hl * DH:(hl + 1) * DH] = Wq[:, h * DH:(h + 1) * DH] * scale
            wqk[:, 1, hl * DH:(hl + 1) * DH] = Wk[:, h * DH:(h + 1) * DH]
        wv = np.ascontiguousarray(
            np.concatenate([Wv[:, (h0 + hl) * DH:(h0 + hl + 1) * DH] for hl in range(HPC)], axis=1))
        wout = np.ascontiguousarray(W_out[h0 * DH:(h0 + HPC) * DH, :])
        in_maps.append({"xT": xT, "wqk": wqk, "wv": wv, "wout": wout,
                        "eb": np.ascontiguousarray(eb_all[h0:h0 + HPC]), "km": km,
                        "idf": np.eye(P, dtype=np.float16),
                        "onesr": np.ones((1, P), np.float32),
                        "vinit": vinit})
    return in_maps


def kernel(x, mask, positions_bias, W_qkv, W_out, b_out, _trace=False):
    from concourse.bass_utils import run_bass_kernel_spmd
    nc = _get_nc()
    in_maps = _prep_inputs(x, mask, positions_bias, W_qkv, W_out)
    res = run_bass_kernel_spmd(nc, in_maps, list(range(N_CORES)), trace=_trace)
    if _trace:
        _nc_cache["last_result"] = res
    outp_t = np.zeros((B, NB, DIM // CI, P, CI), np.float32)
    attnT_t = np.zeros((B, NT, P, CI), np.float32)
    for r in res.results:
        outp_t += r["outp"]
        attnT_t += r["attnT"].astype(np.float32)
    outp = outp_t.transpose(0, 1, 3, 2, 4).reshape(B, N, DIM)
    outp += np.asarray(b_out, np.float32)
    attn_avg = np.zeros((B, N, N), np.float32)
    for t, (ci, jb) in enumerate(TILES):
        attn_avg[:, ci * CI:(ci + 1) * CI, jb * P:(jb + 1) * P] = \
            attnT_t[:, t].transpose(0, 2, 1)
    attn_avg /= np.float32(H)
    return np.ascontiguousarray(outp), attn_avg
